# revision 53
# baseline (speedup 1.0000x reference)
"""EdgePredictionHead on 8 TRN2 NeuronCores.

Sharding: graph-level data parallel — 32 molecules / 8 cores = 4 molecules
per core. Host does the cheap node-level prep (s-projection, coords
centering, per-edge distance, weight folding) and the molecule sharding;
the device kernel runs the dominant edge-level pipeline per core:

    pre^T = W_bond0^T @ e_sym^T  (+)  G^T     (G = a_i + a_j + d*w_d + b_eff)
    h     = silu(pre)
    out^T = W_b1^T @ h                         (b_b1 added on host)

Key structural optimization: the whole computation is edge-pair symmetric
(e_sym, d, and a_i+a_j are all invariant under (j,i) -> (i,j)), so only the
E/2 unique node pairs are computed — exactly 496 pairs per 32-atom molecule,
one 496-wide chunk per molecule, 4 chunks per core — and the host mirrors
the result to both edge directions.

All streams are fp16 ([feat, edges] feature-major so the PE contracts over
partitions at 1 cycle/row). The G-add is fused into the same PSUM bank via an
identity-matrix matmul accumulation; silu runs on ACT straight out of PSUM.
Per-chunk outputs [5, 496] accumulate into disjoint partition rows of a
single PSUM bank (via chunk-padded W_b1 stationaries), drained once at the
end. Dummy matmuls during the input-DMA window pre-ramp the PE p-state, and
an early dummy silu pulls the ACT table load off the critical path.
"""

import os
import sys
import numpy as np

sys.path.insert(0, "/opt/trn_rl_repo")

import concourse.bacc as bacc
import concourse.mybir as mybir
from concourse.tile import TileContext
from concourse.bass_utils import run_bass_kernel_spmd

N_CORES = 8
SDIM = 256
EDIM = 128
NB = 5
ATOMS = 32
PAIRS = ATOMS * (ATOMS - 1) // 2   # 496 unique pairs per molecule
MOL_PER_CORE = 4
E_UNIQ = MOL_PER_CORE * PAIRS      # 1984 unique pairs per core
CH = 496                           # chunk = one molecule's pairs (1 PSUM bank)
NCH = MOL_PER_CORE                 # 4 chunks
WA = 256 + 2                       # stage-1 weights: W_bond0 | b_eff halves
WB1C = NCH * NB                    # 20: padded wb1 cols (chunk ch -> 5ch..5ch+4)
WB = 2 * NCH * WB1C                # 160: the 8 chunk-padded wb1 blocks
KBD = ATOMS + 3                    # 35: K of the G-matmul (nodes + 3 coord rows)
ABDC = NCH * 2 * 128               # 1024: a'-block lhsT cols (chunk x half x 128)
PATC = NCH * CH                    # 1984: pattern+coord rhs cols (chunk x 496)
# DRAM layout (cols): [wt_a | es0] [abd|pat rows 0:35] [wtb] [es1] [es2 es3].
# G is not streamed at all: per chunk it is reconstructed in PSUM by a
# K=35 matmul  lhsT=[a'-block; -2*w_d x3], rhs=[pair-pattern; c_u*c_v rows]
# accumulated on top of the W_bond0^T @ e_sym^T matmul.
NCOL = WA + CH + (ABDC + PATC) + WB + (NCH - 1) * CH

F16 = mybir.dt.float16
F32 = mybir.dt.float32

_nc_cache = {}


def _build_nc():
    if "nc" in _nc_cache:
        return _nc_cache["nc"]
    nc = bacc.Bacc()
    peb = nc.dram_tensor("peb", [128, NCOL], F16, kind="ExternalInput")
    outD = nc.dram_tensor("outD", [NCH * NB, CH], F32, kind="ExternalOutput")

    with TileContext(nc) as tc:
        with tc.tile_pool(name="cst", bufs=1) as cpool, \
             tc.tile_pool(name="hbuf", bufs=NCH) as hpool, \
             tc.tile_pool(name="psA", bufs=3, space="PSUM") as ppA, \
             tc.tile_pool(name="psO", bufs=1, space="PSUM") as ppO:
            # DMA order = critical-path order: [stage-1 weights + es0],
            # [abd/pat block], es1, [wtb], es2+es3. Bacc's
            # generate_event_semaphores splits any excess sync waits, so
            # DMA count is not constrained.
            t0 = cpool.tile([128, WA + CH], F16, tag="t0")
            nc.sync.dma_start(out=t0[:], in_=peb[:, 0:WA + CH])
            c1 = WA + CH
            BDW = 2 * 128 + CH     # 752: one chunk's [abd | pat] block
            abd = cpool.tile([KBD, NCH * BDW], F16, tag="abd")
            nc.sync.dma_start(
                out=abd[:, 0:BDW], in_=peb[0:KBD, c1:c1 + BDW])
            nc.sync.dma_start(
                out=abd[:, BDW:], in_=peb[0:KBD, c1 + BDW:c1 + NCH * BDW])
            c2 = c1 + NCH * BDW
            wtb = cpool.tile([128, WB], F16, tag="wtb")
            es1 = cpool.tile([128, CH], F16, tag="es1")
            nc.sync.dma_start(out=es1[:], in_=peb[:, c2 + WB:c2 + WB + CH])
            nc.sync.dma_start(out=wtb[:], in_=peb[:, c2:c2 + WB])
            es23 = cpool.tile([128, 2 * CH], F16, tag="es23")
            nc.sync.dma_start(
                out=es23[:], in_=peb[:, c2 + WB + CH:c2 + WB + 3 * CH])
            ess = [t0[:, WA:WA + CH], es1[:],
                   es23[:, 0:CH], es23[:, CH:2 * CH]]
            psbig = ppO.tile([NCH * NB, CH], F32, tag="po")

            # warmups, all off a zeroed scratch tile (no DMA dependency):
            #  - ACT silu: pulls the Silu act-table load forward so it
            #    overlaps the input DMA instead of stalling the first silu
            #  - PE: dense dummy matmuls start the tensor-engine p-state
            #    ramp (~3us to full clock) during the DMA window; their
            #    garbage output lands in psbig as a closed accumulation
            #    group that chunk 0's start=True group later overwrites
            scratch = cpool.tile([128, CH], F16, tag="scratch")
            nc.gpsimd.memset(scratch[:], 0.0)
            wu_a = cpool.tile([128, 1], F32, tag="wu_a")
            nc.scalar.activation(
                wu_a[:], scratch[:, 0:1], mybir.ActivationFunctionType.Silu)
            NWU = 5
            for i in range(NWU):
                nc.tensor.matmul(psbig[:], scratch[:, 0:WB1C], scratch[:],
                                 start=(i == 0), stop=(i == NWU - 1))

            Wb = (t0[:, 0:128], t0[:, 128:256])
            bias = (t0[:, 256:257], t0[:, 257:258])

            def bd(ch, hf):
                c0 = ch * BDW + hf * 128
                return abd[:, c0:c0 + 128]

            def pat(ch):
                c0 = ch * BDW + 256
                return abd[:, c0:c0 + CH]

            def wb1pad(ch, hf):
                c0 = (2 * ch + hf) * WB1C
                return wtb[:, c0:c0 + WB1C]

            hs = [None] * NCH

            def stage1(ch):
                # per half: pre^T = W_bond0h^T @ e_sym^T (K=128) + G^T via
                # the K=35 a'/pattern matmul, accumulated in one PSUM bank;
                # silu reads PSUM directly with b_eff as the ACT bias.
                es = ess[ch]
                ps0 = ppA.tile([128, CH], F32, tag="ps0")
                ps1 = ppA.tile([128, CH], F32, tag="ps1")
                nc.tensor.matmul(ps0[:], Wb[0], es, start=True, stop=False)
                nc.tensor.matmul(ps0[:], bd(ch, 0), pat(ch),
                                 start=False, stop=True)
                nc.tensor.matmul(ps1[:], bd(ch, 1), pat(ch),
                                 start=True, stop=False)
                nc.tensor.matmul(ps1[:], Wb[1], es, start=False, stop=True)
                h0 = hpool.tile([128, CH], F16, tag="h0")
                nc.scalar.activation(
                    h0[:], ps0[:], mybir.ActivationFunctionType.Silu,
                    bias=bias[0])
                h1 = hpool.tile([128, CH], F16, tag="h1")
                nc.scalar.activation(
                    h1[:], ps1[:], mybir.ActivationFunctionType.Silu,
                    bias=bias[1])
                hs[ch] = (h0, h1)

            def stage2(ch):
                # chunk ch's padded wb1 is nonzero only in rows 5ch..5ch+4 of
                # the output, so all 8 matmuls accumulate disjoint row
                # blocks of one shared PSUM bank (one group spanning all).
                h0, h1 = hs[ch]
                nc.tensor.matmul(psbig[:], wb1pad(ch, 0), h0[:],
                                 start=(ch == 0), stop=False)
                nc.tensor.matmul(psbig[:], wb1pad(ch, 1), h1[:],
                                 start=False, stop=(ch == NCH - 1))

            for ch in range(NCH):
                stage1(ch)
                if ch >= 1:
                    stage2(ch - 1)
            stage2(NCH - 1)

            # final drain: split the PSUM->SBUF copy across ACT and DVE
            # (copy shares the already-loaded silu act-table set, so the
            # ACT half needs no extra LoadActFuncSet)
            ob = cpool.tile([NCH * NB, CH], F32, tag="ob")
            HC = CH // 2
            nc.scalar.copy(ob[:, 0:HC], psbig[:, 0:HC])
            nc.vector.tensor_copy(ob[:, HC:], psbig[:, HC:])
            nc.sync.dma_start(out=outD[:], in_=ob[:])

    nc.finalize()
    _nc_cache["nc"] = nc
    return nc


def _silu(x):
    return x / (1.0 + np.exp(-x))


def _host_prep(s, v, p, e, batch, edge_index,
               W_shared, b_shared, W_coords, W_bond, b_bond,
               W_b0, b_b0, W_b1, b_b1):
    """Cheap node-level prep + weight folding."""
    n = s.shape[0]
    E = edge_index.shape[1]
    j, i = edge_index[0].astype(np.int64), edge_index[1].astype(np.int64)

    s1 = _silu(s @ W_shared + b_shared)                       # [n, SDIM]
    W0 = np.asarray(W_b0[:SDIM], np.float32)                  # [SDIM, SDIM]
    w_d = np.asarray(W_b0[SDIM], np.float32)                  # [SDIM]
    a = s1 @ W0                                               # [n, SDIM]

    coords = p + (v @ W_coords).reshape(n, 3)
    nmol = int(batch.max()) + 1
    sums = np.zeros((nmol, 3), np.float32)
    np.add.at(sums, batch, coords)
    counts = np.maximum(np.bincount(batch, minlength=nmol), 1).astype(np.float32)
    coords = coords - (sums / counts[:, None])[batch]

    # reverse-edge lookup for symmetrization (0 where reverse edge absent)
    key = j * n + i
    order = np.argsort(key)
    skey = key[order]
    pos = np.clip(np.searchsorted(skey, i * n + j), 0, E - 1)
    rev = order[pos]
    has_rev = skey[pos] == i * n + j
    e_rev = np.where(has_rev[:, None], e[rev], 0.0).astype(np.float32)
    e_sym = 0.5 * (e + e_rev)

    b_eff = (b_bond @ W0 + b_b0).astype(np.float32)           # [SDIM]
    W_bond0 = (W_bond @ W0).astype(np.float32)                # [EDIM, SDIM]
    return a, coords, e_sym, W_bond0, w_d, b_eff, j, i, nmol


def kernel(s, v, p, e, batch, edge_index,
           W_shared, b_shared, W_coords, W_bond, b_bond,
           W_b0, b_b0, W_b1, b_b1):
    s = np.asarray(s, np.float32)
    v = np.asarray(v, np.float32)
    p = np.asarray(p, np.float32)
    e = np.asarray(e, np.float32)
    batch = np.asarray(batch, np.int32)
    edge_index = np.asarray(edge_index, np.int32)
    E = edge_index.shape[1]

    a, coords, e_sym, W_bond0, w_d, b_eff, j, i, nmol = _host_prep(
        s, v, p, e, batch, edge_index, W_shared, b_shared, W_coords,
        W_bond, b_bond, W_b0, b_b0, W_b1, b_b1)
    W_b1 = np.asarray(W_b1, np.float32)
    b_b1 = np.asarray(b_b1, np.float32)

    try:
        # ---- device path: requires the fully-connected intra-molecule
        # structure (every ordered pair (j,i), j!=i, within each molecule) ----
        assert nmol == N_CORES * MOL_PER_CORE
        assert E == nmol * ATOMS * (ATOMS - 1)
        mol = batch[j]
        assert np.array_equal(mol, batch[i])
        lj = j - mol * ATOMS
        li = i - mol * ATOMS
        assert lj.min() >= 0 and lj.max() < ATOMS
        assert li.min() >= 0 and li.max() < ATOMS
        # unique-pair rank within molecule: (u<v) -> prefix(u) + (v-u-1)
        u = np.minimum(lj, li)
        v_ = np.maximum(lj, li)
        assert (u != v_).all()
        rank = (mol * PAIRS + u * (2 * ATOMS - 1 - u) // 2
                + (v_ - u - 1)).astype(np.int64)         # [E] in [0, nmol*PAIRS)
        counts = np.bincount(rank, minlength=nmol * PAIRS)
        assert (counts == 2).all(), "each unordered pair must appear twice"

        # representative edge per unique pair (the j<i direction)
        sel = np.nonzero(lj < li)[0]
        r_sel = rank[sel]
        es_u = np.empty((nmol * PAIRS, EDIM), np.float32)
        es_u[r_sel] = e_sym[sel]

        # G is reconstructed on device:  G[p] = a'_u + a'_v - 2(c_u.c_v) w_d
        # with a' = a + |c|^2 w_d  and  b_eff as the silu bias.
        a1 = (a + (coords ** 2).sum(-1, keepdims=True) * w_d).astype(np.float32)
        cprod_u = np.empty((nmol * PAIRS, 3), np.float32)
        cprod_u[r_sel] = coords[j[sel]] * coords[i[sel]]

        # canonical pair pattern for one 32-atom molecule: [32, 496] 0/1
        uu, vv = np.triu_indices(ATOMS, k=1)
        prank = uu * (2 * ATOMS - 1 - uu) // 2 + (vv - uu - 1)
        pattern = np.zeros((ATOMS, PAIRS), np.float16)
        pattern[uu, prank] = 1.0
        pattern[vv, prank] = 1.0

        wbond16 = W_bond0.astype(np.float16)                   # [128, 256]
        # per-(chunk, half) padded wb1: [128, 20], cols 5ch..5ch+4 filled
        wb1h = (W_b1[:128].astype(np.float16), W_b1[128:].astype(np.float16))
        wb1blk = np.zeros((NCH, 2, 128, WB1C), np.float16)
        for ch in range(NCH):
            for hf in (0, 1):
                wb1blk[ch, hf, :, NB * ch:NB * (ch + 1)] = wb1h[hf]
        wb1cols = wb1blk.transpose(2, 0, 1, 3).reshape(128, 2 * NCH * WB1C)
        wd16 = (-2.0 * w_d).astype(np.float16)                 # [256]
        in_maps = []
        for c in range(N_CORES):
            r0 = c * E_UNIQ
            esT = es_u[r0:r0 + E_UNIQ].astype(np.float16).T    # [128, E_UNIQ]
            peb = np.zeros((128, NCOL), np.float16)
            peb[:, 0:256] = wbond16
            peb[:, 256] = b_eff[:128]
            peb[:, 257] = b_eff[128:]
            peb[:, WA:WA + CH] = esT[:, 0:CH]
            c1 = WA + CH
            BDW = 2 * 128 + CH
            for ch in range(NCH):
                m = c * NCH + ch
                blk = c1 + BDW * ch
                peb[0:ATOMS, blk:blk + 256] = a1[m * ATOMS:(m + 1) * ATOMS]
                peb[ATOMS:KBD, blk:blk + 256] = wd16
                peb[0:ATOMS, blk + 256:blk + BDW] = pattern
                peb[ATOMS:KBD, blk + 256:blk + BDW] = (
                    cprod_u[m * PAIRS:(m + 1) * PAIRS].T)
            c2 = c1 + NCH * BDW
            peb[:, c2:c2 + WB] = wb1cols
            peb[:, c2 + WB:c2 + WB + 3 * CH] = esT[:, CH:]
            in_maps.append({"peb": peb})

        nc = _build_nc()
        res = run_bass_kernel_spmd(nc, in_maps, core_ids=list(range(N_CORES)))
        _nc_cache["last_result"] = res
        results = res.results if hasattr(res, "results") else res
        out_u = np.empty((nmol * PAIRS, NB), np.float32)
        for c in range(N_CORES):
            od = results[c]["outD"]                            # [NCH*NB, CH]
            out_u[c * E_UNIQ:(c + 1) * E_UNIQ] = (
                od.reshape(NCH, NB, CH).transpose(0, 2, 1).reshape(E_UNIQ, NB))
        # mirror unique-pair results to both edge directions
        return out_u[rank] + b_b1
    except Exception:
        if os.environ.get("KERNEL_NO_FALLBACK") == "1":
            raise
        # fallback: same math on host (general edge_index)
        d = ((coords[i] - coords[j]) ** 2).sum(-1).astype(np.float32)
        G = a[i] + a[j] + d[:, None] * w_d + b_eff
        h = _silu(e_sym @ W_bond0 + G)
        return (h @ W_b1 + b_b1).astype(np.float32)


# revision 55
# speedup vs baseline: 1.0110x; 1.0110x over previous
"""EdgePredictionHead on 8 TRN2 NeuronCores.

Sharding: graph-level data parallel — 32 molecules / 8 cores = 4 molecules
per core. Host does the cheap node-level prep (s-projection, coords
centering, per-edge distance, weight folding) and the molecule sharding;
the device kernel runs the dominant edge-level pipeline per core:

    pre^T = W_bond0^T @ e_sym^T  (+)  G^T     (G = a_i + a_j + d*w_d + b_eff)
    h     = silu(pre)
    out^T = W_b1^T @ h                         (b_b1 added on host)

Key structural optimization: the whole computation is edge-pair symmetric
(e_sym, d, and a_i+a_j are all invariant under (j,i) -> (i,j)), so only the
E/2 unique node pairs are computed — exactly 496 pairs per 32-atom molecule,
one 496-wide chunk per molecule, 4 chunks per core — and the host mirrors
the result to both edge directions.

All streams are fp16 ([feat, edges] feature-major so the PE contracts over
partitions at 1 cycle/row). The G-add is fused into the same PSUM bank via an
identity-matrix matmul accumulation; silu runs on ACT straight out of PSUM.
Per-chunk outputs [5, 496] accumulate into disjoint partition rows of a
single PSUM bank (via chunk-padded W_b1 stationaries), drained once at the
end. Dummy matmuls during the input-DMA window pre-ramp the PE p-state, and
an early dummy silu pulls the ACT table load off the critical path.
"""

import os
import sys
import numpy as np

sys.path.insert(0, "/opt/trn_rl_repo")

import concourse.bacc as bacc
import concourse.mybir as mybir
from concourse.tile import TileContext
from concourse.bass_utils import run_bass_kernel_spmd

N_CORES = 8
SDIM = 256
EDIM = 128
NB = 5
ATOMS = 32
PAIRS = ATOMS * (ATOMS - 1) // 2   # 496 unique pairs per molecule
MOL_PER_CORE = 4
E_UNIQ = MOL_PER_CORE * PAIRS      # 1984 unique pairs per core
CH = 496                           # chunk = one molecule's pairs (1 PSUM bank)
NCH = MOL_PER_CORE                 # 4 chunks
WA = 256 + 2                       # stage-1 weights: W_bond0 | b_eff halves
WB1C = NCH * NB                    # 20: padded wb1 cols (chunk ch -> 5ch..5ch+4)
WB = 2 * NCH * WB1C                # 160: the 8 chunk-padded wb1 blocks
KBD = ATOMS + 3                    # 35: K of the G-matmul (nodes + 3 coord rows)
ABDC = NCH * 2 * 128               # 1024: a'-block lhsT cols (chunk x half x 128)
PATC = NCH * CH                    # 1984: pattern+coord rhs cols (chunk x 496)
# DRAM layout (cols): [wt_a | es0] [abd|pat rows 0:35] [wtb] [es1] [es2 es3].
# G is not streamed at all: per chunk it is reconstructed in PSUM by a
# K=35 matmul  lhsT=[a'-block; -2*w_d x3], rhs=[pair-pattern; c_u*c_v rows]
# accumulated on top of the W_bond0^T @ e_sym^T matmul.
NCOL = WA + CH + (ABDC + PATC) + WB + (NCH - 1) * CH

F16 = mybir.dt.float16
F32 = mybir.dt.float32

_nc_cache = {}


def _build_nc():
    if "nc" in _nc_cache:
        return _nc_cache["nc"]
    nc = bacc.Bacc()
    peb = nc.dram_tensor("peb", [128, NCOL], F16, kind="ExternalInput")
    outD = nc.dram_tensor("outD", [NCH * NB, CH], F32, kind="ExternalOutput")

    with TileContext(nc) as tc:
        with tc.tile_pool(name="cst", bufs=1) as cpool, \
             tc.tile_pool(name="hbuf", bufs=NCH) as hpool, \
             tc.tile_pool(name="psA", bufs=3, space="PSUM") as ppA, \
             tc.tile_pool(name="psO", bufs=1, space="PSUM") as ppO:
            # DMA order = critical-path order: [stage-1 weights + es0],
            # [abd/pat block], es1, [wtb], es2+es3. Bacc's
            # generate_event_semaphores splits any excess sync waits, so
            # DMA count is not constrained.
            t0 = cpool.tile([128, WA + CH], F16, tag="t0")
            nc.sync.dma_start(out=t0[:], in_=peb[:, 0:WA + CH])
            c1 = WA + CH
            BDW = 2 * 128 + CH     # 752: one chunk's [abd | pat] block
            abd = cpool.tile([KBD, NCH * BDW], F16, tag="abd")

            def dma_abd(ch):
                nc.sync.dma_start(
                    out=abd[:, BDW * ch:BDW * (ch + 1)],
                    in_=peb[0:KBD, c1 + BDW * ch:c1 + BDW * (ch + 1)])

            c2 = c1 + NCH * BDW
            dma_abd(0)
            wtb = cpool.tile([128, WB], F16, tag="wtb")
            es1 = cpool.tile([128, CH], F16, tag="es1")
            nc.sync.dma_start(out=es1[:], in_=peb[:, c2 + WB:c2 + WB + CH])
            dma_abd(1)
            nc.sync.dma_start(out=wtb[:], in_=peb[:, c2:c2 + WB])
            es23 = cpool.tile([128, 2 * CH], F16, tag="es23")
            nc.sync.dma_start(
                out=es23[:], in_=peb[:, c2 + WB + CH:c2 + WB + 3 * CH])
            dma_abd(2)
            dma_abd(3)
            ess = [t0[:, WA:WA + CH], es1[:],
                   es23[:, 0:CH], es23[:, CH:2 * CH]]
            psbig = ppO.tile([NCH * NB, CH], F32, tag="po")

            # warmups, all off a zeroed scratch tile (no DMA dependency):
            #  - ACT silu: pulls the Silu act-table load forward so it
            #    overlaps the input DMA instead of stalling the first silu
            #  - PE: dense dummy matmuls start the tensor-engine p-state
            #    ramp (~3us to full clock) during the DMA window; their
            #    garbage output lands in psbig as a closed accumulation
            #    group that chunk 0's start=True group later overwrites
            scratch = cpool.tile([128, CH], F16, tag="scratch")
            nc.gpsimd.memset(scratch[:], 0.0)
            wu_a = cpool.tile([128, 1], F32, tag="wu_a")
            nc.scalar.activation(
                wu_a[:], scratch[:, 0:1], mybir.ActivationFunctionType.Silu)
            NWU = 5
            for i in range(NWU):
                nc.tensor.matmul(psbig[:], scratch[:, 0:WB1C], scratch[:],
                                 start=(i == 0), stop=(i == NWU - 1))

            Wb = (t0[:, 0:128], t0[:, 128:256])
            bias = (t0[:, 256:257], t0[:, 257:258])

            def bd(ch, hf):
                c0 = ch * BDW + hf * 128
                return abd[:, c0:c0 + 128]

            def pat(ch):
                c0 = ch * BDW + 256
                return abd[:, c0:c0 + CH]

            def wb1pad(ch, hf):
                c0 = (2 * ch + hf) * WB1C
                return wtb[:, c0:c0 + WB1C]

            hs = [None] * NCH

            def stage1(ch):
                # per half: pre^T = W_bond0h^T @ e_sym^T (K=128) + G^T via
                # the K=35 a'/pattern matmul, accumulated in one PSUM bank;
                # silu reads PSUM directly with b_eff as the ACT bias.
                es = ess[ch]
                ps0 = ppA.tile([128, CH], F32, tag="ps0")
                ps1 = ppA.tile([128, CH], F32, tag="ps1")
                nc.tensor.matmul(ps0[:], Wb[0], es, start=True, stop=False)
                nc.tensor.matmul(ps0[:], bd(ch, 0), pat(ch),
                                 start=False, stop=True)
                nc.tensor.matmul(ps1[:], bd(ch, 1), pat(ch),
                                 start=True, stop=False)
                nc.tensor.matmul(ps1[:], Wb[1], es, start=False, stop=True)
                h0 = hpool.tile([128, CH], F16, tag="h0")
                nc.scalar.activation(
                    h0[:], ps0[:], mybir.ActivationFunctionType.Silu,
                    bias=bias[0])
                h1 = hpool.tile([128, CH], F16, tag="h1")
                nc.scalar.activation(
                    h1[:], ps1[:], mybir.ActivationFunctionType.Silu,
                    bias=bias[1])
                hs[ch] = (h0, h1)

            def stage2(ch):
                # chunk ch's padded wb1 is nonzero only in rows 5ch..5ch+4 of
                # the output, so all 8 matmuls accumulate disjoint row
                # blocks of one shared PSUM bank (one group spanning all).
                h0, h1 = hs[ch]
                nc.tensor.matmul(psbig[:], wb1pad(ch, 0), h0[:],
                                 start=(ch == 0), stop=False)
                nc.tensor.matmul(psbig[:], wb1pad(ch, 1), h1[:],
                                 start=False, stop=(ch == NCH - 1))

            for ch in range(NCH):
                stage1(ch)
                if ch >= 1:
                    stage2(ch - 1)
            stage2(NCH - 1)

            # final drain on ACT: copy shares the already-loaded silu
            # act-table set (no extra LoadActFuncSet), and the same-engine
            # dependency on the last silu makes it dispatch ~40ns after the
            # final stage-2 matmul (vs ~240ns for a cross-engine DVE hop)
            ob = cpool.tile([NCH * NB, CH], F32, tag="ob")
            nc.scalar.copy(ob[:], psbig[:])
            nc.sync.dma_start(out=outD[:], in_=ob[:])

    nc.finalize()
    _nc_cache["nc"] = nc
    return nc


def _silu(x):
    return x / (1.0 + np.exp(-x))


def _host_prep(s, v, p, e, batch, edge_index,
               W_shared, b_shared, W_coords, W_bond, b_bond,
               W_b0, b_b0, W_b1, b_b1):
    """Cheap node-level prep + weight folding."""
    n = s.shape[0]
    E = edge_index.shape[1]
    j, i = edge_index[0].astype(np.int64), edge_index[1].astype(np.int64)

    s1 = _silu(s @ W_shared + b_shared)                       # [n, SDIM]
    W0 = np.asarray(W_b0[:SDIM], np.float32)                  # [SDIM, SDIM]
    w_d = np.asarray(W_b0[SDIM], np.float32)                  # [SDIM]
    a = s1 @ W0                                               # [n, SDIM]

    coords = p + (v @ W_coords).reshape(n, 3)
    nmol = int(batch.max()) + 1
    sums = np.zeros((nmol, 3), np.float32)
    np.add.at(sums, batch, coords)
    counts = np.maximum(np.bincount(batch, minlength=nmol), 1).astype(np.float32)
    coords = coords - (sums / counts[:, None])[batch]

    # reverse-edge lookup for symmetrization (0 where reverse edge absent)
    key = j * n + i
    order = np.argsort(key)
    skey = key[order]
    pos = np.clip(np.searchsorted(skey, i * n + j), 0, E - 1)
    rev = order[pos]
    has_rev = skey[pos] == i * n + j
    e_rev = np.where(has_rev[:, None], e[rev], 0.0).astype(np.float32)
    e_sym = 0.5 * (e + e_rev)

    b_eff = (b_bond @ W0 + b_b0).astype(np.float32)           # [SDIM]
    W_bond0 = (W_bond @ W0).astype(np.float32)                # [EDIM, SDIM]
    return a, coords, e_sym, W_bond0, w_d, b_eff, j, i, nmol


def kernel(s, v, p, e, batch, edge_index,
           W_shared, b_shared, W_coords, W_bond, b_bond,
           W_b0, b_b0, W_b1, b_b1):
    s = np.asarray(s, np.float32)
    v = np.asarray(v, np.float32)
    p = np.asarray(p, np.float32)
    e = np.asarray(e, np.float32)
    batch = np.asarray(batch, np.int32)
    edge_index = np.asarray(edge_index, np.int32)
    E = edge_index.shape[1]

    a, coords, e_sym, W_bond0, w_d, b_eff, j, i, nmol = _host_prep(
        s, v, p, e, batch, edge_index, W_shared, b_shared, W_coords,
        W_bond, b_bond, W_b0, b_b0, W_b1, b_b1)
    W_b1 = np.asarray(W_b1, np.float32)
    b_b1 = np.asarray(b_b1, np.float32)

    try:
        # ---- device path: requires the fully-connected intra-molecule
        # structure (every ordered pair (j,i), j!=i, within each molecule) ----
        assert nmol == N_CORES * MOL_PER_CORE
        assert E == nmol * ATOMS * (ATOMS - 1)
        mol = batch[j]
        assert np.array_equal(mol, batch[i])
        lj = j - mol * ATOMS
        li = i - mol * ATOMS
        assert lj.min() >= 0 and lj.max() < ATOMS
        assert li.min() >= 0 and li.max() < ATOMS
        # unique-pair rank within molecule: (u<v) -> prefix(u) + (v-u-1)
        u = np.minimum(lj, li)
        v_ = np.maximum(lj, li)
        assert (u != v_).all()
        rank = (mol * PAIRS + u * (2 * ATOMS - 1 - u) // 2
                + (v_ - u - 1)).astype(np.int64)         # [E] in [0, nmol*PAIRS)
        counts = np.bincount(rank, minlength=nmol * PAIRS)
        assert (counts == 2).all(), "each unordered pair must appear twice"

        # representative edge per unique pair (the j<i direction)
        sel = np.nonzero(lj < li)[0]
        r_sel = rank[sel]
        es_u = np.empty((nmol * PAIRS, EDIM), np.float32)
        es_u[r_sel] = e_sym[sel]

        # G is reconstructed on device:  G[p] = a'_u + a'_v - 2(c_u.c_v) w_d
        # with a' = a + |c|^2 w_d  and  b_eff as the silu bias.
        a1 = (a + (coords ** 2).sum(-1, keepdims=True) * w_d).astype(np.float32)
        cprod_u = np.empty((nmol * PAIRS, 3), np.float32)
        cprod_u[r_sel] = coords[j[sel]] * coords[i[sel]]

        # canonical pair pattern for one 32-atom molecule: [32, 496] 0/1
        uu, vv = np.triu_indices(ATOMS, k=1)
        prank = uu * (2 * ATOMS - 1 - uu) // 2 + (vv - uu - 1)
        pattern = np.zeros((ATOMS, PAIRS), np.float16)
        pattern[uu, prank] = 1.0
        pattern[vv, prank] = 1.0

        wbond16 = W_bond0.astype(np.float16)                   # [128, 256]
        # per-(chunk, half) padded wb1: [128, 20], cols 5ch..5ch+4 filled
        wb1h = (W_b1[:128].astype(np.float16), W_b1[128:].astype(np.float16))
        wb1blk = np.zeros((NCH, 2, 128, WB1C), np.float16)
        for ch in range(NCH):
            for hf in (0, 1):
                wb1blk[ch, hf, :, NB * ch:NB * (ch + 1)] = wb1h[hf]
        wb1cols = wb1blk.transpose(2, 0, 1, 3).reshape(128, 2 * NCH * WB1C)
        wd16 = (-2.0 * w_d).astype(np.float16)                 # [256]
        in_maps = []
        for c in range(N_CORES):
            r0 = c * E_UNIQ
            esT = es_u[r0:r0 + E_UNIQ].astype(np.float16).T    # [128, E_UNIQ]
            peb = np.zeros((128, NCOL), np.float16)
            peb[:, 0:256] = wbond16
            peb[:, 256] = b_eff[:128]
            peb[:, 257] = b_eff[128:]
            peb[:, WA:WA + CH] = esT[:, 0:CH]
            c1 = WA + CH
            BDW = 2 * 128 + CH
            for ch in range(NCH):
                m = c * NCH + ch
                blk = c1 + BDW * ch
                peb[0:ATOMS, blk:blk + 256] = a1[m * ATOMS:(m + 1) * ATOMS]
                peb[ATOMS:KBD, blk:blk + 256] = wd16
                peb[0:ATOMS, blk + 256:blk + BDW] = pattern
                peb[ATOMS:KBD, blk + 256:blk + BDW] = (
                    cprod_u[m * PAIRS:(m + 1) * PAIRS].T)
            c2 = c1 + NCH * BDW
            peb[:, c2:c2 + WB] = wb1cols
            peb[:, c2 + WB:c2 + WB + 3 * CH] = esT[:, CH:]
            in_maps.append({"peb": peb})

        nc = _build_nc()
        res = run_bass_kernel_spmd(nc, in_maps, core_ids=list(range(N_CORES)))
        _nc_cache["last_result"] = res
        results = res.results if hasattr(res, "results") else res
        out_u = np.empty((nmol * PAIRS, NB), np.float32)
        for c in range(N_CORES):
            od = results[c]["outD"]                            # [NCH*NB, CH]
            out_u[c * E_UNIQ:(c + 1) * E_UNIQ] = (
                od.reshape(NCH, NB, CH).transpose(0, 2, 1).reshape(E_UNIQ, NB))
        # mirror unique-pair results to both edge directions
        return out_u[rank] + b_b1
    except Exception:
        if os.environ.get("KERNEL_NO_FALLBACK") == "1":
            raise
        # fallback: same math on host (general edge_index)
        d = ((coords[i] - coords[j]) ** 2).sum(-1).astype(np.float32)
        G = a[i] + a[j] + d[:, None] * w_d + b_eff
        h = _silu(e_sym @ W_bond0 + G)
        return (h @ W_b1 + b_b1).astype(np.float32)


# revision 56
# speedup vs baseline: 1.0383x; 1.0269x over previous
"""EdgePredictionHead on 8 TRN2 NeuronCores.

Sharding: graph-level data parallel — 32 molecules / 8 cores = 4 molecules
per core. Host does the cheap node-level prep (s-projection, coords
centering, per-edge distance, weight folding) and the molecule sharding;
the device kernel runs the dominant edge-level pipeline per core:

    pre^T = W_bond0^T @ e_sym^T  (+)  G^T     (G = a_i + a_j + d*w_d + b_eff)
    h     = silu(pre)
    out^T = W_b1^T @ h                         (b_b1 added on host)

Key structural optimization: the whole computation is edge-pair symmetric
(e_sym, d, and a_i+a_j are all invariant under (j,i) -> (i,j)), so only the
E/2 unique node pairs are computed — exactly 496 pairs per 32-atom molecule,
one 496-wide chunk per molecule, 4 chunks per core — and the host mirrors
the result to both edge directions.

All streams are fp16 ([feat, edges] feature-major so the PE contracts over
partitions at 1 cycle/row). The G-add is fused into the same PSUM bank via an
identity-matrix matmul accumulation; silu runs on ACT straight out of PSUM.
Per-chunk outputs [5, 496] accumulate into disjoint partition rows of a
single PSUM bank (via chunk-padded W_b1 stationaries), drained once at the
end. Dummy matmuls during the input-DMA window pre-ramp the PE p-state, and
an early dummy silu pulls the ACT table load off the critical path.
"""

import os
import sys
import numpy as np

sys.path.insert(0, "/opt/trn_rl_repo")

import concourse.bacc as bacc
import concourse.mybir as mybir
from concourse.tile import TileContext
from concourse.bass_utils import run_bass_kernel_spmd

N_CORES = 8
SDIM = 256
EDIM = 128
NB = 5
ATOMS = 32
PAIRS = ATOMS * (ATOMS - 1) // 2   # 496 unique pairs per molecule
MOL_PER_CORE = 4
E_UNIQ = MOL_PER_CORE * PAIRS      # 1984 unique pairs per core
CH = 496                           # chunk = one molecule's pairs (1 PSUM bank)
NCH = MOL_PER_CORE                 # 4 chunks
WA = 256 + 2                       # stage-1 weights: W_bond0 | b_eff halves
WB1C = NCH * NB                    # 20: padded wb1 cols (chunk ch -> 5ch..5ch+4)
WB = 2 * NCH * WB1C                # 160: the 8 chunk-padded wb1 blocks
KBD = ATOMS + 3                    # 35: K of the G-matmul (nodes + 3 coord rows)
ABDC = NCH * 2 * 128               # 1024: a'-block lhsT cols (chunk x half x 128)
PATC = NCH * CH                    # 1984: pattern+coord rhs cols (chunk x 496)
# DRAM layout (cols): [wt_a | es0] [abd|pat rows 0:35] [wtb] [es1] [es2 es3].
# G is not streamed at all: per chunk it is reconstructed in PSUM by a
# K=35 matmul  lhsT=[a'-block; -2*w_d x3], rhs=[pair-pattern; c_u*c_v rows]
# accumulated on top of the W_bond0^T @ e_sym^T matmul.
NCOL = WA + CH + (ABDC + PATC) + WB + (NCH - 1) * CH

F16 = mybir.dt.float16
F32 = mybir.dt.float32

_nc_cache = {}


def _build_nc():
    if "nc" in _nc_cache:
        return _nc_cache["nc"]
    nc = bacc.Bacc()
    peb = nc.dram_tensor("peb", [128, NCOL], F16, kind="ExternalInput")
    outD = nc.dram_tensor("outD", [NCH * NB, CH], F32, kind="ExternalOutput")

    with TileContext(nc) as tc:
        with tc.tile_pool(name="cst", bufs=1) as cpool, \
             tc.tile_pool(name="hbuf", bufs=NCH) as hpool, \
             tc.tile_pool(name="psA", bufs=3, space="PSUM") as ppA, \
             tc.tile_pool(name="psO", bufs=1, space="PSUM") as ppO:
            # DMA order = critical-path order: [stage-1 weights + es0],
            # [abd/pat block], es1, [wtb], es2+es3. Bacc's
            # generate_event_semaphores splits any excess sync waits, so
            # DMA count is not constrained.
            t0 = cpool.tile([128, WA + CH], F16, tag="t0")
            nc.sync.dma_start(out=t0[:], in_=peb[:, 0:WA + CH])
            c1 = WA + CH
            BDW = 2 * 128 + CH     # 752: one chunk's [abd | pat] block
            abd = cpool.tile([KBD, NCH * BDW], F16, tag="abd")

            def dma_abd(ch):
                nc.sync.dma_start(
                    out=abd[:, BDW * ch:BDW * (ch + 1)],
                    in_=peb[0:KBD, c1 + BDW * ch:c1 + BDW * (ch + 1)])

            c2 = c1 + NCH * BDW
            dma_abd(0)
            wtb = cpool.tile([128, WB], F16, tag="wtb")
            es1 = cpool.tile([128, CH], F16, tag="es1")
            nc.sync.dma_start(out=es1[:], in_=peb[:, c2 + WB:c2 + WB + CH])
            dma_abd(1)
            nc.sync.dma_start(out=wtb[:], in_=peb[:, c2:c2 + WB])
            es23 = cpool.tile([128, 2 * CH], F16, tag="es23")
            nc.sync.dma_start(
                out=es23[:, 0:CH], in_=peb[:, c2 + WB + CH:c2 + WB + 2 * CH])
            dma_abd(2)
            nc.sync.dma_start(
                out=es23[:, CH:], in_=peb[:, c2 + WB + 2 * CH:c2 + WB + 3 * CH])
            dma_abd(3)
            ess = [t0[:, WA:WA + CH], es1[:],
                   es23[:, 0:CH], es23[:, CH:2 * CH]]
            psbig = ppO.tile([NCH * NB, CH], F32, tag="po")

            # warmups, all off a zeroed scratch tile (no DMA dependency):
            #  - ACT silu: pulls the Silu act-table load forward so it
            #    overlaps the input DMA instead of stalling the first silu
            #  - PE: dense dummy matmuls start the tensor-engine p-state
            #    ramp (~3us to full clock) during the DMA window; their
            #    garbage output lands in psbig as a closed accumulation
            #    group that chunk 0's start=True group later overwrites
            scratch = cpool.tile([128, CH], F16, tag="scratch")
            nc.gpsimd.memset(scratch[:], 0.0)
            wu_a = cpool.tile([128, 1], F32, tag="wu_a")
            nc.scalar.activation(
                wu_a[:], scratch[:, 0:1], mybir.ActivationFunctionType.Silu)
            NWU = 5
            for i in range(NWU):
                nc.tensor.matmul(psbig[:], scratch[:, 0:WB1C], scratch[:],
                                 start=(i == 0), stop=(i == NWU - 1))

            Wb = (t0[:, 0:128], t0[:, 128:256])
            bias = (t0[:, 256:257], t0[:, 257:258])

            def bd(ch, hf):
                c0 = ch * BDW + hf * 128
                return abd[:, c0:c0 + 128]

            def pat(ch):
                c0 = ch * BDW + 256
                return abd[:, c0:c0 + CH]

            def wb1pad(ch, hf):
                c0 = (2 * ch + hf) * WB1C
                return wtb[:, c0:c0 + WB1C]

            hs = [None] * NCH

            def stage1(ch):
                # per half: pre^T = W_bond0h^T @ e_sym^T (K=128) + G^T via
                # the K=35 a'/pattern matmul, accumulated in one PSUM bank;
                # silu reads PSUM directly with b_eff as the ACT bias.
                es = ess[ch]
                ps0 = ppA.tile([128, CH], F32, tag="ps0")
                ps1 = ppA.tile([128, CH], F32, tag="ps1")
                nc.tensor.matmul(ps0[:], Wb[0], es, start=True, stop=False)
                nc.tensor.matmul(ps0[:], bd(ch, 0), pat(ch),
                                 start=False, stop=True)
                nc.tensor.matmul(ps1[:], bd(ch, 1), pat(ch),
                                 start=True, stop=False)
                nc.tensor.matmul(ps1[:], Wb[1], es, start=False, stop=True)
                h0 = hpool.tile([128, CH], F16, tag="h0")
                nc.scalar.activation(
                    h0[:], ps0[:], mybir.ActivationFunctionType.Silu,
                    bias=bias[0])
                h1 = hpool.tile([128, CH], F16, tag="h1")
                nc.scalar.activation(
                    h1[:], ps1[:], mybir.ActivationFunctionType.Silu,
                    bias=bias[1])
                hs[ch] = (h0, h1)

            def stage2(ch):
                # chunk ch's padded wb1 is nonzero only in rows 5ch..5ch+4 of
                # the output, so all 8 matmuls accumulate disjoint row
                # blocks of one shared PSUM bank (one group spanning all).
                h0, h1 = hs[ch]
                nc.tensor.matmul(psbig[:], wb1pad(ch, 0), h0[:],
                                 start=(ch == 0), stop=False)
                nc.tensor.matmul(psbig[:], wb1pad(ch, 1), h1[:],
                                 start=False, stop=(ch == NCH - 1))

            for ch in range(NCH):
                stage1(ch)
                if ch >= 1:
                    stage2(ch - 1)
            stage2(NCH - 1)

            # final drain on ACT: copy shares the already-loaded silu
            # act-table set (no extra LoadActFuncSet), and the same-engine
            # dependency on the last silu makes it dispatch ~40ns after the
            # final stage-2 matmul (vs ~240ns for a cross-engine DVE hop)
            ob = cpool.tile([NCH * NB, CH], F32, tag="ob")
            nc.scalar.copy(ob[:], psbig[:])
            nc.sync.dma_start(out=outD[:], in_=ob[:])

    nc.finalize()
    _nc_cache["nc"] = nc
    return nc


def _silu(x):
    return x / (1.0 + np.exp(-x))


def _host_prep(s, v, p, e, batch, edge_index,
               W_shared, b_shared, W_coords, W_bond, b_bond,
               W_b0, b_b0, W_b1, b_b1):
    """Cheap node-level prep + weight folding."""
    n = s.shape[0]
    E = edge_index.shape[1]
    j, i = edge_index[0].astype(np.int64), edge_index[1].astype(np.int64)

    s1 = _silu(s @ W_shared + b_shared)                       # [n, SDIM]
    W0 = np.asarray(W_b0[:SDIM], np.float32)                  # [SDIM, SDIM]
    w_d = np.asarray(W_b0[SDIM], np.float32)                  # [SDIM]
    a = s1 @ W0                                               # [n, SDIM]

    coords = p + (v @ W_coords).reshape(n, 3)
    nmol = int(batch.max()) + 1
    sums = np.zeros((nmol, 3), np.float32)
    np.add.at(sums, batch, coords)
    counts = np.maximum(np.bincount(batch, minlength=nmol), 1).astype(np.float32)
    coords = coords - (sums / counts[:, None])[batch]

    # reverse-edge lookup for symmetrization (0 where reverse edge absent)
    key = j * n + i
    order = np.argsort(key)
    skey = key[order]
    pos = np.clip(np.searchsorted(skey, i * n + j), 0, E - 1)
    rev = order[pos]
    has_rev = skey[pos] == i * n + j
    e_rev = np.where(has_rev[:, None], e[rev], 0.0).astype(np.float32)
    e_sym = 0.5 * (e + e_rev)

    b_eff = (b_bond @ W0 + b_b0).astype(np.float32)           # [SDIM]
    W_bond0 = (W_bond @ W0).astype(np.float32)                # [EDIM, SDIM]
    return a, coords, e_sym, W_bond0, w_d, b_eff, j, i, nmol


def kernel(s, v, p, e, batch, edge_index,
           W_shared, b_shared, W_coords, W_bond, b_bond,
           W_b0, b_b0, W_b1, b_b1):
    s = np.asarray(s, np.float32)
    v = np.asarray(v, np.float32)
    p = np.asarray(p, np.float32)
    e = np.asarray(e, np.float32)
    batch = np.asarray(batch, np.int32)
    edge_index = np.asarray(edge_index, np.int32)
    E = edge_index.shape[1]

    a, coords, e_sym, W_bond0, w_d, b_eff, j, i, nmol = _host_prep(
        s, v, p, e, batch, edge_index, W_shared, b_shared, W_coords,
        W_bond, b_bond, W_b0, b_b0, W_b1, b_b1)
    W_b1 = np.asarray(W_b1, np.float32)
    b_b1 = np.asarray(b_b1, np.float32)

    try:
        # ---- device path: requires the fully-connected intra-molecule
        # structure (every ordered pair (j,i), j!=i, within each molecule) ----
        assert nmol == N_CORES * MOL_PER_CORE
        assert E == nmol * ATOMS * (ATOMS - 1)
        mol = batch[j]
        assert np.array_equal(mol, batch[i])
        lj = j - mol * ATOMS
        li = i - mol * ATOMS
        assert lj.min() >= 0 and lj.max() < ATOMS
        assert li.min() >= 0 and li.max() < ATOMS
        # unique-pair rank within molecule: (u<v) -> prefix(u) + (v-u-1)
        u = np.minimum(lj, li)
        v_ = np.maximum(lj, li)
        assert (u != v_).all()
        rank = (mol * PAIRS + u * (2 * ATOMS - 1 - u) // 2
                + (v_ - u - 1)).astype(np.int64)         # [E] in [0, nmol*PAIRS)
        counts = np.bincount(rank, minlength=nmol * PAIRS)
        assert (counts == 2).all(), "each unordered pair must appear twice"

        # representative edge per unique pair (the j<i direction)
        sel = np.nonzero(lj < li)[0]
        r_sel = rank[sel]
        es_u = np.empty((nmol * PAIRS, EDIM), np.float32)
        es_u[r_sel] = e_sym[sel]

        # G is reconstructed on device:  G[p] = a'_u + a'_v - 2(c_u.c_v) w_d
        # with a' = a + |c|^2 w_d  and  b_eff as the silu bias.
        a1 = (a + (coords ** 2).sum(-1, keepdims=True) * w_d).astype(np.float32)
        cprod_u = np.empty((nmol * PAIRS, 3), np.float32)
        cprod_u[r_sel] = coords[j[sel]] * coords[i[sel]]

        # canonical pair pattern for one 32-atom molecule: [32, 496] 0/1
        uu, vv = np.triu_indices(ATOMS, k=1)
        prank = uu * (2 * ATOMS - 1 - uu) // 2 + (vv - uu - 1)
        pattern = np.zeros((ATOMS, PAIRS), np.float16)
        pattern[uu, prank] = 1.0
        pattern[vv, prank] = 1.0

        wbond16 = W_bond0.astype(np.float16)                   # [128, 256]
        # per-(chunk, half) padded wb1: [128, 20], cols 5ch..5ch+4 filled
        wb1h = (W_b1[:128].astype(np.float16), W_b1[128:].astype(np.float16))
        wb1blk = np.zeros((NCH, 2, 128, WB1C), np.float16)
        for ch in range(NCH):
            for hf in (0, 1):
                wb1blk[ch, hf, :, NB * ch:NB * (ch + 1)] = wb1h[hf]
        wb1cols = wb1blk.transpose(2, 0, 1, 3).reshape(128, 2 * NCH * WB1C)
        wd16 = (-2.0 * w_d).astype(np.float16)                 # [256]
        in_maps = []
        for c in range(N_CORES):
            r0 = c * E_UNIQ
            esT = es_u[r0:r0 + E_UNIQ].astype(np.float16).T    # [128, E_UNIQ]
            peb = np.zeros((128, NCOL), np.float16)
            peb[:, 0:256] = wbond16
            peb[:, 256] = b_eff[:128]
            peb[:, 257] = b_eff[128:]
            peb[:, WA:WA + CH] = esT[:, 0:CH]
            c1 = WA + CH
            BDW = 2 * 128 + CH
            for ch in range(NCH):
                m = c * NCH + ch
                blk = c1 + BDW * ch
                peb[0:ATOMS, blk:blk + 256] = a1[m * ATOMS:(m + 1) * ATOMS]
                peb[ATOMS:KBD, blk:blk + 256] = wd16
                peb[0:ATOMS, blk + 256:blk + BDW] = pattern
                peb[ATOMS:KBD, blk + 256:blk + BDW] = (
                    cprod_u[m * PAIRS:(m + 1) * PAIRS].T)
            c2 = c1 + NCH * BDW
            peb[:, c2:c2 + WB] = wb1cols
            peb[:, c2 + WB:c2 + WB + 3 * CH] = esT[:, CH:]
            in_maps.append({"peb": peb})

        nc = _build_nc()
        res = run_bass_kernel_spmd(nc, in_maps, core_ids=list(range(N_CORES)))
        _nc_cache["last_result"] = res
        results = res.results if hasattr(res, "results") else res
        out_u = np.empty((nmol * PAIRS, NB), np.float32)
        for c in range(N_CORES):
            od = results[c]["outD"]                            # [NCH*NB, CH]
            out_u[c * E_UNIQ:(c + 1) * E_UNIQ] = (
                od.reshape(NCH, NB, CH).transpose(0, 2, 1).reshape(E_UNIQ, NB))
        # mirror unique-pair results to both edge directions
        return out_u[rank] + b_b1
    except Exception:
        if os.environ.get("KERNEL_NO_FALLBACK") == "1":
            raise
        # fallback: same math on host (general edge_index)
        d = ((coords[i] - coords[j]) ** 2).sum(-1).astype(np.float32)
        G = a[i] + a[j] + d[:, None] * w_d + b_eff
        h = _silu(e_sym @ W_bond0 + G)
        return (h @ W_b1 + b_b1).astype(np.float32)


# revision 57
# speedup vs baseline: 1.0742x; 1.0346x over previous
"""EdgePredictionHead on 8 TRN2 NeuronCores.

Sharding: graph-level data parallel — 32 molecules / 8 cores = 4 molecules
per core. Host does the cheap node-level prep (s-projection, coords
centering, per-edge distance, weight folding) and the molecule sharding;
the device kernel runs the dominant edge-level pipeline per core:

    pre^T = W_bond0^T @ e_sym^T  (+)  G^T     (G = a_i + a_j + d*w_d + b_eff)
    h     = silu(pre)
    out^T = W_b1^T @ h                         (b_b1 added on host)

Key structural optimization: the whole computation is edge-pair symmetric
(e_sym, d, and a_i+a_j are all invariant under (j,i) -> (i,j)), so only the
E/2 unique node pairs are computed — exactly 496 pairs per 32-atom molecule,
one 496-wide chunk per molecule, 4 chunks per core — and the host mirrors
the result to both edge directions.

All streams are fp16 ([feat, edges] feature-major so the PE contracts over
partitions at 1 cycle/row). The G-add is fused into the same PSUM bank via an
identity-matrix matmul accumulation; silu runs on ACT straight out of PSUM.
Per-chunk outputs [5, 496] accumulate into disjoint partition rows of a
single PSUM bank (via chunk-padded W_b1 stationaries), drained once at the
end. Dummy matmuls during the input-DMA window pre-ramp the PE p-state, and
an early dummy silu pulls the ACT table load off the critical path.
"""

import os
import sys
import numpy as np

sys.path.insert(0, "/opt/trn_rl_repo")

import concourse.bacc as bacc
import concourse.mybir as mybir
from concourse.tile import TileContext
from concourse.bass_utils import run_bass_kernel_spmd

N_CORES = 8
SDIM = 256
EDIM = 128
NB = 5
ATOMS = 32
PAIRS = ATOMS * (ATOMS - 1) // 2   # 496 unique pairs per molecule
MOL_PER_CORE = 4
E_UNIQ = MOL_PER_CORE * PAIRS      # 1984 unique pairs per core
CH = 496                           # chunk = one molecule's pairs (1 PSUM bank)
NCH = MOL_PER_CORE                 # 4 chunks
WA = 256 + 2                       # stage-1 weights: W_bond0 | b_eff halves
WB1C = NCH * NB                    # 20: padded wb1 cols (chunk ch -> 5ch..5ch+4)
WB = 2 * NCH * WB1C                # 160: the 8 chunk-padded wb1 blocks
KBD = ATOMS + 3                    # 35: K of the G-matmul (nodes + 3 coord rows)
ABDC = NCH * 2 * 128               # 1024: a'-block lhsT cols (chunk x half x 128)
PATC = NCH * CH                    # 1984: pattern+coord rhs cols (chunk x 496)
# DRAM layout (cols): [wt_a | es0] [abd|pat rows 0:35] [wtb] [es1] [es2 es3].
# G is not streamed at all: per chunk it is reconstructed in PSUM by a
# K=35 matmul  lhsT=[a'-block; -2*w_d x3], rhs=[pair-pattern; c_u*c_v rows]
# accumulated on top of the W_bond0^T @ e_sym^T matmul.
NCOL = WA + CH + (ABDC + PATC) + WB + (NCH - 1) * CH

F16 = mybir.dt.float16
F32 = mybir.dt.float32

_nc_cache = {}


def _build_nc():
    if "nc" in _nc_cache:
        return _nc_cache["nc"]
    nc = bacc.Bacc()
    peb = nc.dram_tensor("peb", [128, NCOL], F16, kind="ExternalInput")
    outD = nc.dram_tensor("outD", [NCH * NB, CH], F32, kind="ExternalOutput")

    with TileContext(nc) as tc:
        with tc.tile_pool(name="cst", bufs=1) as cpool, \
             tc.tile_pool(name="hbuf", bufs=NCH) as hpool, \
             tc.tile_pool(name="psA", bufs=3, space="PSUM") as ppA, \
             tc.tile_pool(name="psO", bufs=1, space="PSUM") as ppO:
            # DMA order = critical-path order: [stage-1 weights + es0],
            # [abd/pat block], es1, [wtb], es2+es3. Bacc's
            # generate_event_semaphores splits any excess sync waits, so
            # DMA count is not constrained.
            t0 = cpool.tile([128, WA + CH], F16, tag="t0")
            nc.sync.dma_start(out=t0[:], in_=peb[:, 0:WA + CH])
            c1 = WA + CH
            BDW = 2 * 128 + CH     # 752: one chunk's [abd | pat] block
            abd = cpool.tile([KBD, NCH * BDW], F16, tag="abd")

            def dma_abd(ch):
                nc.sync.dma_start(
                    out=abd[:, BDW * ch:BDW * (ch + 1)],
                    in_=peb[0:KBD, c1 + BDW * ch:c1 + BDW * (ch + 1)])

            c2 = c1 + NCH * BDW
            nc.sync.dma_start(
                out=abd[:], in_=peb[0:KBD, c1:c1 + NCH * BDW])
            wtb = cpool.tile([128, WB], F16, tag="wtb")
            es1 = cpool.tile([128, CH], F16, tag="es1")
            nc.sync.dma_start(out=es1[:], in_=peb[:, c2 + WB:c2 + WB + CH])
            nc.sync.dma_start(out=wtb[:], in_=peb[:, c2:c2 + WB])
            es23 = cpool.tile([128, 2 * CH], F16, tag="es23")
            nc.sync.dma_start(
                out=es23[:], in_=peb[:, c2 + WB + CH:c2 + WB + 3 * CH])
            ess = [t0[:, WA:WA + CH], es1[:],
                   es23[:, 0:CH], es23[:, CH:2 * CH]]
            psbig = ppO.tile([NCH * NB, CH], F32, tag="po")

            # warmups, all off a zeroed scratch tile (no DMA dependency):
            #  - ACT silu: pulls the Silu act-table load forward so it
            #    overlaps the input DMA instead of stalling the first silu
            #  - PE: dense dummy matmuls start the tensor-engine p-state
            #    ramp (~3us to full clock) during the DMA window; their
            #    garbage output lands in psbig as a closed accumulation
            #    group that chunk 0's start=True group later overwrites
            scratch = cpool.tile([128, CH], F16, tag="scratch")
            nc.gpsimd.memset(scratch[:], 0.0)
            wu_a = cpool.tile([128, 1], F32, tag="wu_a")
            nc.scalar.activation(
                wu_a[:], scratch[:, 0:1], mybir.ActivationFunctionType.Silu)
            NWU = 5
            for i in range(NWU):
                nc.tensor.matmul(psbig[:], scratch[:, 0:WB1C], scratch[:],
                                 start=(i == 0), stop=(i == NWU - 1))

            Wb = (t0[:, 0:128], t0[:, 128:256])
            bias = (t0[:, 256:257], t0[:, 257:258])

            def bd(ch, hf):
                c0 = ch * BDW + hf * 128
                return abd[:, c0:c0 + 128]

            def pat(ch):
                c0 = ch * BDW + 256
                return abd[:, c0:c0 + CH]

            def wb1pad(ch, hf):
                c0 = (2 * ch + hf) * WB1C
                return wtb[:, c0:c0 + WB1C]

            hs = [None] * NCH

            def stage1(ch):
                # per half: pre^T = W_bond0h^T @ e_sym^T (K=128) + G^T via
                # the K=35 a'/pattern matmul, accumulated in one PSUM bank;
                # silu reads PSUM directly with b_eff as the ACT bias.
                es = ess[ch]
                ps0 = ppA.tile([128, CH], F32, tag="ps0")
                ps1 = ppA.tile([128, CH], F32, tag="ps1")
                nc.tensor.matmul(ps0[:], Wb[0], es, start=True, stop=False)
                nc.tensor.matmul(ps0[:], bd(ch, 0), pat(ch),
                                 start=False, stop=True)
                nc.tensor.matmul(ps1[:], bd(ch, 1), pat(ch),
                                 start=True, stop=False)
                nc.tensor.matmul(ps1[:], Wb[1], es, start=False, stop=True)
                h0 = hpool.tile([128, CH], F16, tag="h0")
                nc.scalar.activation(
                    h0[:], ps0[:], mybir.ActivationFunctionType.Silu,
                    bias=bias[0])
                h1 = hpool.tile([128, CH], F16, tag="h1")
                nc.scalar.activation(
                    h1[:], ps1[:], mybir.ActivationFunctionType.Silu,
                    bias=bias[1])
                hs[ch] = (h0, h1)

            def stage2(ch):
                # chunk ch's padded wb1 is nonzero only in rows 5ch..5ch+4 of
                # the output, so all 8 matmuls accumulate disjoint row
                # blocks of one shared PSUM bank (one group spanning all).
                h0, h1 = hs[ch]
                nc.tensor.matmul(psbig[:], wb1pad(ch, 0), h0[:],
                                 start=(ch == 0), stop=False)
                nc.tensor.matmul(psbig[:], wb1pad(ch, 1), h1[:],
                                 start=False, stop=(ch == NCH - 1))

            for ch in range(NCH):
                stage1(ch)
                if ch >= 1:
                    stage2(ch - 1)
            stage2(NCH - 1)

            # final drain on ACT: copy shares the already-loaded silu
            # act-table set (no extra LoadActFuncSet), and the same-engine
            # dependency on the last silu makes it dispatch ~40ns after the
            # final stage-2 matmul (vs ~240ns for a cross-engine DVE hop)
            ob = cpool.tile([NCH * NB, CH], F32, tag="ob")
            nc.scalar.copy(ob[:], psbig[:])
            nc.sync.dma_start(out=outD[:], in_=ob[:])

    nc.finalize()
    _nc_cache["nc"] = nc
    return nc


def _silu(x):
    return x / (1.0 + np.exp(-x))


def _host_prep(s, v, p, e, batch, edge_index,
               W_shared, b_shared, W_coords, W_bond, b_bond,
               W_b0, b_b0, W_b1, b_b1):
    """Cheap node-level prep + weight folding."""
    n = s.shape[0]
    E = edge_index.shape[1]
    j, i = edge_index[0].astype(np.int64), edge_index[1].astype(np.int64)

    s1 = _silu(s @ W_shared + b_shared)                       # [n, SDIM]
    W0 = np.asarray(W_b0[:SDIM], np.float32)                  # [SDIM, SDIM]
    w_d = np.asarray(W_b0[SDIM], np.float32)                  # [SDIM]
    a = s1 @ W0                                               # [n, SDIM]

    coords = p + (v @ W_coords).reshape(n, 3)
    nmol = int(batch.max()) + 1
    sums = np.zeros((nmol, 3), np.float32)
    np.add.at(sums, batch, coords)
    counts = np.maximum(np.bincount(batch, minlength=nmol), 1).astype(np.float32)
    coords = coords - (sums / counts[:, None])[batch]

    # reverse-edge lookup for symmetrization (0 where reverse edge absent)
    key = j * n + i
    order = np.argsort(key)
    skey = key[order]
    pos = np.clip(np.searchsorted(skey, i * n + j), 0, E - 1)
    rev = order[pos]
    has_rev = skey[pos] == i * n + j
    e_rev = np.where(has_rev[:, None], e[rev], 0.0).astype(np.float32)
    e_sym = 0.5 * (e + e_rev)

    b_eff = (b_bond @ W0 + b_b0).astype(np.float32)           # [SDIM]
    W_bond0 = (W_bond @ W0).astype(np.float32)                # [EDIM, SDIM]
    return a, coords, e_sym, W_bond0, w_d, b_eff, j, i, nmol


def kernel(s, v, p, e, batch, edge_index,
           W_shared, b_shared, W_coords, W_bond, b_bond,
           W_b0, b_b0, W_b1, b_b1):
    s = np.asarray(s, np.float32)
    v = np.asarray(v, np.float32)
    p = np.asarray(p, np.float32)
    e = np.asarray(e, np.float32)
    batch = np.asarray(batch, np.int32)
    edge_index = np.asarray(edge_index, np.int32)
    E = edge_index.shape[1]

    a, coords, e_sym, W_bond0, w_d, b_eff, j, i, nmol = _host_prep(
        s, v, p, e, batch, edge_index, W_shared, b_shared, W_coords,
        W_bond, b_bond, W_b0, b_b0, W_b1, b_b1)
    W_b1 = np.asarray(W_b1, np.float32)
    b_b1 = np.asarray(b_b1, np.float32)

    try:
        # ---- device path: requires the fully-connected intra-molecule
        # structure (every ordered pair (j,i), j!=i, within each molecule) ----
        assert nmol == N_CORES * MOL_PER_CORE
        assert E == nmol * ATOMS * (ATOMS - 1)
        mol = batch[j]
        assert np.array_equal(mol, batch[i])
        lj = j - mol * ATOMS
        li = i - mol * ATOMS
        assert lj.min() >= 0 and lj.max() < ATOMS
        assert li.min() >= 0 and li.max() < ATOMS
        # unique-pair rank within molecule: (u<v) -> prefix(u) + (v-u-1)
        u = np.minimum(lj, li)
        v_ = np.maximum(lj, li)
        assert (u != v_).all()
        rank = (mol * PAIRS + u * (2 * ATOMS - 1 - u) // 2
                + (v_ - u - 1)).astype(np.int64)         # [E] in [0, nmol*PAIRS)
        counts = np.bincount(rank, minlength=nmol * PAIRS)
        assert (counts == 2).all(), "each unordered pair must appear twice"

        # representative edge per unique pair (the j<i direction)
        sel = np.nonzero(lj < li)[0]
        r_sel = rank[sel]
        es_u = np.empty((nmol * PAIRS, EDIM), np.float32)
        es_u[r_sel] = e_sym[sel]

        # G is reconstructed on device:  G[p] = a'_u + a'_v - 2(c_u.c_v) w_d
        # with a' = a + |c|^2 w_d  and  b_eff as the silu bias.
        a1 = (a + (coords ** 2).sum(-1, keepdims=True) * w_d).astype(np.float32)
        cprod_u = np.empty((nmol * PAIRS, 3), np.float32)
        cprod_u[r_sel] = coords[j[sel]] * coords[i[sel]]

        # canonical pair pattern for one 32-atom molecule: [32, 496] 0/1
        uu, vv = np.triu_indices(ATOMS, k=1)
        prank = uu * (2 * ATOMS - 1 - uu) // 2 + (vv - uu - 1)
        pattern = np.zeros((ATOMS, PAIRS), np.float16)
        pattern[uu, prank] = 1.0
        pattern[vv, prank] = 1.0

        wbond16 = W_bond0.astype(np.float16)                   # [128, 256]
        # per-(chunk, half) padded wb1: [128, 20], cols 5ch..5ch+4 filled
        wb1h = (W_b1[:128].astype(np.float16), W_b1[128:].astype(np.float16))
        wb1blk = np.zeros((NCH, 2, 128, WB1C), np.float16)
        for ch in range(NCH):
            for hf in (0, 1):
                wb1blk[ch, hf, :, NB * ch:NB * (ch + 1)] = wb1h[hf]
        wb1cols = wb1blk.transpose(2, 0, 1, 3).reshape(128, 2 * NCH * WB1C)
        wd16 = (-2.0 * w_d).astype(np.float16)                 # [256]
        in_maps = []
        for c in range(N_CORES):
            r0 = c * E_UNIQ
            esT = es_u[r0:r0 + E_UNIQ].astype(np.float16).T    # [128, E_UNIQ]
            peb = np.zeros((128, NCOL), np.float16)
            peb[:, 0:256] = wbond16
            peb[:, 256] = b_eff[:128]
            peb[:, 257] = b_eff[128:]
            peb[:, WA:WA + CH] = esT[:, 0:CH]
            c1 = WA + CH
            BDW = 2 * 128 + CH
            for ch in range(NCH):
                m = c * NCH + ch
                blk = c1 + BDW * ch
                peb[0:ATOMS, blk:blk + 256] = a1[m * ATOMS:(m + 1) * ATOMS]
                peb[ATOMS:KBD, blk:blk + 256] = wd16
                peb[0:ATOMS, blk + 256:blk + BDW] = pattern
                peb[ATOMS:KBD, blk + 256:blk + BDW] = (
                    cprod_u[m * PAIRS:(m + 1) * PAIRS].T)
            c2 = c1 + NCH * BDW
            peb[:, c2:c2 + WB] = wb1cols
            peb[:, c2 + WB:c2 + WB + 3 * CH] = esT[:, CH:]
            in_maps.append({"peb": peb})

        nc = _build_nc()
        res = run_bass_kernel_spmd(nc, in_maps, core_ids=list(range(N_CORES)))
        _nc_cache["last_result"] = res
        results = res.results if hasattr(res, "results") else res
        out_u = np.empty((nmol * PAIRS, NB), np.float32)
        for c in range(N_CORES):
            od = results[c]["outD"]                            # [NCH*NB, CH]
            out_u[c * E_UNIQ:(c + 1) * E_UNIQ] = (
                od.reshape(NCH, NB, CH).transpose(0, 2, 1).reshape(E_UNIQ, NB))
        # mirror unique-pair results to both edge directions
        return out_u[rank] + b_b1
    except Exception:
        if os.environ.get("KERNEL_NO_FALLBACK") == "1":
            raise
        # fallback: same math on host (general edge_index)
        d = ((coords[i] - coords[j]) ** 2).sum(-1).astype(np.float32)
        G = a[i] + a[j] + d[:, None] * w_d + b_eff
        h = _silu(e_sym @ W_bond0 + G)
        return (h @ W_b1 + b_b1).astype(np.float32)
